# revision 27
# baseline (speedup 1.0000x reference)
"""Causal self-attention (RoPE) Trainium2 kernel, 8-core SPMD, single launch.

Sharding: tensor-parallel over heads. Core i owns heads (2i, 2i+1):
  - qkv projection slice (bf16), RoPE, causal attention, and a PARTIAL output
    projection over its own 128 y-features. Host sums the 8 partials and adds
    the bias (both linear, so they commute with the per-core split).

Design notes:
  - All matmuls are bf16 (fp32r runs in 4-cycle/row HIGH mode on HW).
  - q/k kept feature-major [dim, tok]; head dims permuted to [evens|odds] so
    RoPE rotate-half becomes a signed 32-row block swap, done on the PE with a
    small permutation matmul (PT) instead of SBUF->SBUF DMAs.
  - v computed directly token-major [tok, dim] (x-tile as lhsT), with a ones
    column appended per head so the AV matmul also produces softmax
    denominators for free (row 64 of ys).
  - Scores are [k, q] with the two heads' 512-wide strips packed side by side
    in one 2-bank PSUM strip -> ONE wide exp instruction per (kt, window) on
    the Activation engine (its throughput is the second roofline after PE).
  - Causal masking of diagonal blocks: Pool-engine affine_select zeroing the
    upper triangle of probs (exp is computed unmasked; scores ~ N(0,1)).
  - Normalization: reciprocal of the denominator rows, broadcast across the
    128 feature partitions with a tiny PE matmul (pat2), applied to y during
    the PSUM->SBUF copy, BEFORE the output projection (scale commutes).
  - Projection/out-projection matmuls are interleaved into the attention kt
    loop as "filler" so the PE never head-of-line blocks on exp.
"""

import math
from collections import deque

import numpy as np
import ml_dtypes

import concourse.bass as bass
import concourse.tile as tile
from concourse import bacc, mybir
from concourse.bass_utils import run_bass_kernel_spmd
from concourse.masks import make_identity

F32 = mybir.dt.float32
BF16 = mybir.dt.bfloat16

B, T, C = 2, 2048, 1024
H, D = 16, 64
NCORES = 8
HPC = H // NCORES          # heads per core (2)
BT = B * T                 # 4096 tokens
TCH = 512                  # token chunk = q window
NKT = C // 128             # contraction tiles over C (8)
NCH = BT // TCH            # chunks (8)
KPW = TCH // 128           # k-tiles per window step (4)


def build_kernel(debug=False):
    nc = bacc.Bacc("TRN2", target_bir_lowering=False, debug=False,
                   num_devices=NCORES)
    xT_d = nc.dram_tensor("xT", [C, BT], BF16, kind="ExternalInput").ap()
    wq_d = nc.dram_tensor("wq", [C, 128], BF16, kind="ExternalInput").ap()
    wk_d = nc.dram_tensor("wk", [C, 128], BF16, kind="ExternalInput").ap()
    wv_d = nc.dram_tensor("wv", [C, 128], BF16, kind="ExternalInput").ap()
    cos_d = nc.dram_tensor("cosT", [128, T], BF16, kind="ExternalInput").ap()
    sin_d = nc.dram_tensor("sinT", [128, T], BF16, kind="ExternalInput").ap()
    pat_d = nc.dram_tensor("pat33", [33, 128], BF16, kind="ExternalInput").ap()
    wp_d = nc.dram_tensor("wp", [128, C], BF16, kind="ExternalInput").ap()
    out_d = nc.dram_tensor("partial", [BT, C], BF16, kind="ExternalOutput").ap()
    rdram = nc.dram_tensor("rscratch", [HPC, TCH], BF16)
    if debug:
        qrot_d = nc.dram_tensor("qrot_d", [128, BT], BF16, kind="ExternalOutput").ap()
        krot_d = nc.dram_tensor("krot_d", [128, BT], BF16, kind="ExternalOutput").ap()
        vnat_d = nc.dram_tensor("vnat_d", [128, BT // 128, HPC, 65], BF16,
                                kind="ExternalOutput").ap()
        ysb_d = nc.dram_tensor("ysb_d", [NCH, 128, TCH], BF16, kind="ExternalOutput").ap()
        rr_d = nc.dram_tensor("rr_d", [NCH, HPC, TCH], BF16, kind="ExternalOutput").ap()
        pr_d = nc.dram_tensor("pr_d", [16, 128, 2, TCH], BF16, kind="ExternalOutput").ap()
        brec_d = nc.dram_tensor("brec_d", [NCH, 128, TCH], BF16, kind="ExternalOutput").ap()

    with tile.TileContext(nc) as tc:
        with tc.tile_pool(name="consts", bufs=1) as consts, \
             tc.tile_pool(name="persist", bufs=1) as persist, \
             tc.tile_pool(name="xinp", bufs=2) as xinp, \
             tc.tile_pool(name="work", bufs=2) as work, \
             tc.tile_pool(name="probsp", bufs=3) as probsp, \
             tc.tile_pool(name="ps", bufs=1, space="PSUM") as ps:

            # ---------------- constants ----------------
            wq_t = consts.tile([128, NKT, 128], BF16, tag="wq")
            wk_t = consts.tile([128, NKT, 128], BF16, tag="wk")
            wv_t = consts.tile([128, NKT, 128], BF16, tag="wv")
            nc.sync.dma_start(out=wq_t[:], in_=wq_d.rearrange("(kt p) c -> p kt c", p=128))
            nc.sync.dma_start(out=wk_t[:], in_=wk_d.rearrange("(kt p) c -> p kt c", p=128))
            nc.sync.dma_start(out=wv_t[:], in_=wv_d.rearrange("(kt p) c -> p kt c", p=128))
            cos_t = consts.tile([128, T], BF16, tag="cos")
            sin_t = consts.tile([128, T], BF16, tag="sin")
            nc.sync.dma_start(out=cos_t[:], in_=cos_d)
            nc.sync.dma_start(out=sin_t[:], in_=sin_d)
            pat_t = consts.tile([33, 128], BF16, tag="pat")
            nc.sync.dma_start(out=pat_t[:], in_=pat_d)
            wp_t = consts.tile([128, C], BF16, tag="wp")
            nc.sync.dma_start(out=wp_t[:], in_=wp_d)
            ident = consts.tile([128, 128], BF16, tag="ident")
            make_identity(nc, ident)

            # ---------------- persistent ----------------
            qrot = persist.tile([128, BT], BF16, tag="qrot")
            krot = persist.tile([128, BT], BF16, tag="krot")
            # v token-major: [tok, ktile, head, dim|one]
            vnat = persist.tile([128, BT // 128, HPC, 65], BF16, tag="vnat")
            nc.gpsimd.memset(vnat[:, :, :, 64:65], 1.0)
            # denominator reciprocals land on rows 0 and 32 (32-aligned engine
            # bases); other rows stay zero so the pat33 broadcast matmul
            # (contraction 33) reproduces each half from its row.
            rT = persist.tile([33, TCH], BF16, tag="rT")
            nc.gpsimd.memset(rT[:, :], 0.0)

            scale = float(1.0 / math.sqrt(D))

            # ---------------- helpers ----------------
            def proj_ops(c):
                """Filler closures computing qkv projection + RoPE of chunk c."""
                g0 = c * TCH
                t0 = g0 % T
                ops = []

                def dma_x():
                    xt = xinp.tile([128, NKT, TCH], BF16, tag="xt", name=f"xt{c}")
                    nc.sync.dma_start(
                        out=xt[:],
                        in_=xT_d.rearrange("(kt p) t -> p kt t", p=128)[:, :, g0:g0 + TCH])
                    return xt
                xt_box = {}
                ops.append(lambda: xt_box.__setitem__("t", dma_x()))

                def qk_mm(wt, half, box, tag):
                    def f():
                        if half == 0:
                            box["ps"] = ps.tile([128, TCH], F32, tag="gen", bufs=2,
                                                name=f"{tag}ps{c}")
                        pp = box["ps"]
                        for kt in range(4 * half, 4 * half + 4):
                            nc.tensor.matmul(pp[:], wt[:, kt, :], xt_box["t"][:, kt, :],
                                             start=(kt == 0), stop=(kt == 7))
                    return f

                def qk_rope(box, dest, tag):
                    def f():
                        raw = work.tile([128, TCH], BF16, tag="raw", name=f"raw{tag}{c}")
                        nc.scalar.copy(raw[:], box["ps"][:])
                        # rotate-half: swap 32-row blocks via SBUF->SBUF DMA
                        # (sign carried by the host-prepped signed sin rows)
                        sh = work.tile([128, TCH], BF16, tag="sh", name=f"sh{tag}{c}")
                        for blk in range(4):
                            src = blk ^ 1
                            nc.sync.dma_start(out=sh[32 * blk:32 * (blk + 1), :],
                                              in_=raw[32 * src:32 * (src + 1), :])
                        tmp = work.tile([128, TCH], BF16, tag="tmp", name=f"tmp{tag}{c}")
                        nc.vector.tensor_mul(tmp[:], sh[:], sin_t[:, t0:t0 + TCH])
                        dst = dest[:, g0:g0 + TCH]
                        nc.vector.tensor_mul(dst, raw[:], cos_t[:, t0:t0 + TCH])
                        nc.vector.tensor_add(dst, dst, tmp[:])
                    return f

                # NOTE: every "gen"-ring PSUM tile must be consumed before two
                # more gen allocations happen (ring bufs=2) — keep each
                # producer's reader within the next closure.
                qb, kb = {}, {}
                ops.append(qk_mm(wq_t, 0, qb, "q"))
                ops.append(qk_mm(wq_t, 1, qb, "q"))
                ops.append(qk_rope(qb, qrot, "q"))
                ops.append(qk_mm(wk_t, 0, kb, "k"))
                ops.append(qk_mm(wk_t, 1, kb, "k"))

                vb = {}

                def v_mm(half):
                    def f():
                        if half == 0:
                            vb["ps"] = ps.tile([128, TCH], F32, tag="gen", bufs=2,
                                               name=f"vps{c}")
                        pp = vb["ps"]
                        for kt in range(4 * half, 4 * half + 4):
                            nc.tensor.matmul(pp[:], wv_t[:, kt, :], xt_box["t"][:, kt, :],
                                             start=(kt == 0), stop=(kt == 7))
                    return f

                def v_stage():
                    vstage = work.tile([128, TCH], BF16, tag="vstage", name=f"vst{c}")
                    nc.vector.tensor_copy(vstage[:], vb["ps"][:])
                    vb["st"] = vstage

                def v_tr(m):
                    def f():
                        trp = ps.tile([128, 128], BF16, tag="gen", bufs=2,
                                      name=f"vtr{c}_{m}")
                        nc.tensor.transpose(trp[:], vb["st"][:, 128 * m:128 * (m + 1)],
                                            ident[:])
                        vt = c * KPW + m
                        for h in range(HPC):
                            nc.vector.tensor_copy(vnat[:, vt, h, 0:64],
                                                  trp[:, 64 * h:64 * h + 64])
                    return f

                ops.append(qk_rope(kb, krot, "k"))
                ops.append(v_mm(0))
                ops.append(v_mm(1))
                ops.append(v_stage)
                ops.append(v_tr(0))
                ops.append(v_tr(1))
                ops.append(v_tr(2))
                ops.append(v_tr(3))
                return ops

            def outproj_ops(c, ybox):
                """Filler closures projecting normalized y chunk c through wp."""
                g0 = c * TCH
                ops = []

                def otile(m):
                    def f():
                        ysb = ybox["ysb"]
                        for n in range(2):
                            po = ps.tile([128, TCH], F32, tag="gen", bufs=2,
                                         name=f"po{c}_{m}_{n}")
                            nc.tensor.matmul(po[:], ysb[:, 128 * m:128 * (m + 1)],
                                             wp_t[:, TCH * n:TCH * (n + 1)],
                                             start=True, stop=True)
                            ostage = work.tile([128, TCH], BF16, tag="ostage",
                                               bufs=3, name=f"os{c}_{m}_{n}")
                            nc.vector.tensor_copy(ostage[:], po[:])
                            nc.sync.dma_start(
                                out=out_d[g0 + 128 * m:g0 + 128 * (m + 1),
                                          TCH * n:TCH * (n + 1)],
                                in_=ostage[:])
                    return f
                for m in range(4):
                    ops.append(otile(m))
                return ops

            # ---------------- main pipeline ----------------
            queue = deque(proj_ops(0))
            while queue:  # chunk 0 projection up-front
                queue.popleft()()

            prev_outproj = []
            for c in range(NCH):
                b, wl = divmod(c, T // TCH)
                q0l = wl * TCH
                gq = c * TCH
                nkt = KPW * (wl + 1)

                if c + 1 < NCH:
                    queue.extend(proj_ops(c + 1))
                queue.extend(prev_outproj)
                prev_outproj = []

                ys = [ps.tile([65, TCH], F32, tag=f"ys{h}", bufs=1, name=f"ys{h}_{c}")
                      for h in range(HPC)]
                strips = {}
                probs = {}

                def sc(kt):
                    k0l = 128 * kt
                    js = max(k0l - q0l, 0)
                    st = ps.tile([128, 2, TCH], F32, tag="strip", bufs=2,
                                 name=f"st{c}_{kt}")
                    strips[kt] = st
                    for h in range(HPC):
                        hp = 64 * h
                        nc.tensor.matmul(
                            st[:, h, js:TCH],
                            krot[hp:hp + 64, b * T + k0l:b * T + k0l + 128],
                            qrot[hp:hp + 64, gq + js:gq + TCH],
                            start=True, stop=True, tile_position=(hp, 0))
                    pr = probsp.tile([128, 2, TCH], BF16, tag="pr", name=f"pr{c}_{kt}")
                    probs[kt] = pr
                    nc.scalar.activation(pr[:, :, js:TCH], st[:, :, js:TCH],
                                         mybir.ActivationFunctionType.Exp,
                                         scale=scale)
                    if js > 0 or kt == q0l // 128:
                        # diagonal block: zero probs where q < k, i.e. keep
                        # j - p >= 0 (p = key partition, j = query column)
                        nc.gpsimd.affine_select(
                            out=pr[:, :, js:js + 128], in_=pr[:, :, js:js + 128],
                            compare_op=mybir.AluOpType.is_ge, fill=0.0,
                            base=0, pattern=[[0, 2], [1, 128]],
                            channel_multiplier=-1)
                    if debug and c == NCH - 1:
                        nc.sync.dma_start(out=pr_d[kt], in_=pr[:, :, :])

                def av(kt):
                    k0l = 128 * kt
                    js = max(k0l - q0l, 0)
                    vt = b * (T // 128) + kt
                    for h in range(HPC):
                        nc.tensor.matmul(ys[h][:, js:TCH],
                                         vnat[:, vt, h, :],
                                         probs[kt][:, h, js:TCH],
                                         start=(kt == 0), stop=(kt == nkt - 1))
                    del probs[kt], strips[kt]

                sc(0)
                if nkt > 1:
                    sc(1)
                for kt in range(nkt):
                    ndrain = -(-len(queue) // (nkt - kt))
                    for _ in range(min(ndrain, len(queue))):
                        queue.popleft()()
                    av(kt)
                    if kt + 2 < nkt:
                        sc(kt + 2)

                # normalization deferred into the next window's filler queue so
                # the recip chain's latency hides under the next scores/exp.
                def norm_ops(c, ys):
                    box = {}

                    def n1():
                        # 1/d: copy denom rows to SBUF fp32, 1-op DVE approx
                        # reciprocal (18-bit), convert to bf16 rT rows.
                        for h in range(HPC):
                            dsb = work.tile([1, TCH], F32, tag=f"d{h}",
                                            name=f"d{h}_{c}")
                            nc.vector.tensor_copy(dsb[0:1, :], ys[h][64:65, :])
                            rf = work.tile([1, TCH], F32, tag=f"rf{h}",
                                           name=f"rf{h}_{c}")
                            nc.vector.reciprocal_approx_fast(rf[0:1, :], dsb[0:1, :])
                            with nc.allow_low_precision(reason="recip bf16 ok"):
                                nc.vector.tensor_copy(rT[32 * h:32 * h + 1, :],
                                                      rf[0:1, :])
                            nc.sync.dma_start(out=rdram.ap()[h:h + 1, :],
                                              in_=rT[32 * h:32 * h + 1, :])
                        if debug:
                            for h in range(HPC):
                                nc.sync.dma_start(out=rr_d[c, h:h + 1, :],
                                                  in_=rT[32 * h:32 * h + 1, :])

                    def n2():
                        brec = work.tile([128, TCH], BF16, tag="brec",
                                         name=f"brec{c}")
                        for h in range(HPC):
                            row = rdram.ap()[h:h + 1, :]
                            bcast = bass.AP(tensor=row.tensor, offset=row.offset,
                                            ap=[[0, 64]] + list(row.ap)[1:])
                            nc.sync.dma_start(out=brec[64 * h:64 * h + 64, :],
                                              in_=bcast)
                        box["brec"] = brec
                        if debug:
                            nc.sync.dma_start(out=brec_d[c], in_=brec[:])

                    def n3():
                        brec = box["brec"]
                        ysb = work.tile([128, TCH], BF16, tag="ysb",
                                        name=f"ysb{c}")
                        for h in range(HPC):
                            nc.vector.tensor_mul(ysb[64 * h:64 * h + 64, :],
                                                 ys[h][0:64, :],
                                                 brec[64 * h:64 * h + 64, :])
                        box["ysb"] = ysb
                        if debug:
                            nc.sync.dma_start(out=ysb_d[c], in_=ysb[:])
                    return [n1, n2, n3], box

                nops, ybox = norm_ops(c, ys)
                prev_outproj = nops + outproj_ops(c, ybox)

            while queue:
                queue.popleft()()
            for op in prev_outproj:
                op()
            if debug:
                nc.sync.dma_start(out=qrot_d, in_=qrot[:])
                nc.sync.dma_start(out=krot_d, in_=krot[:])
                nc.sync.dma_start(out=vnat_d, in_=vnat[:, :, :, :])

    nc.compile()
    return nc


def _host_prep(x, w_qkv):
    bf16 = ml_dtypes.bfloat16
    xT = np.ascontiguousarray(x.reshape(BT, C).T).astype(bf16)  # [C, BT]
    perm = np.concatenate([np.arange(0, D, 2), np.arange(1, D, 2)])
    inv = 1.0 / (10000.0 ** (np.arange(0, D, 2, dtype=np.float64) / D))
    f = np.outer(np.arange(T, dtype=np.float64), inv)  # [T, 32]
    cosT = np.cos(f).T
    sinT = np.sin(f).T
    C128 = np.ascontiguousarray(np.concatenate([cosT] * 4, 0)).astype(bf16)
    S128 = np.ascontiguousarray(
        np.concatenate([-sinT, sinT, -sinT, sinT], 0)).astype(bf16)

    in_maps = []
    for i in range(NCORES):
        h0, h1 = HPC * i, HPC * i + 1
        wq = np.concatenate([w_qkv[:, h0 * D + perm], w_qkv[:, h1 * D + perm]], 1)
        wk = np.concatenate([w_qkv[:, C + h0 * D + perm], w_qkv[:, C + h1 * D + perm]], 1)
        wv = np.concatenate([w_qkv[:, 2 * C + h0 * D:2 * C + (h0 + 1) * D],
                             w_qkv[:, 2 * C + h1 * D:2 * C + (h1 + 1) * D]], 1)
        pat33 = np.zeros((33, 128), dtype=np.float32)
        pat33[0, 0:64] = 1.0
        pat33[32, 64:128] = 1.0
        in_maps.append({
            "xT": xT, "pat33": pat33.astype(bf16),
            "wq": np.ascontiguousarray(wq).astype(bf16),
            "wk": np.ascontiguousarray(wk).astype(bf16),
            "wv": np.ascontiguousarray(wv).astype(bf16),
            "cosT": C128, "sinT": S128,
        })
    return in_maps


_CACHE = {}


def _get_kernel():
    if "k" not in _CACHE:
        _CACHE["k"] = build_kernel()
    return _CACHE["k"]


def run(x, w_qkv, w_proj, b_proj, trace=False, tmpdirs=(None,), debug=False):
    if debug:
        ncb = build_kernel(debug=True)
    else:
        ncb = _get_kernel()
    x = np.asarray(x)
    w_qkv = np.asarray(w_qkv)
    w_proj = np.asarray(w_proj)
    b_proj = np.asarray(b_proj)
    in_maps = _host_prep(x, w_qkv)
    bf16 = ml_dtypes.bfloat16
    for i in range(NCORES):
        h0 = HPC * i
        wp = np.ascontiguousarray(w_proj[h0 * D:(h0 + HPC) * D]).astype(bf16)
        in_maps[i]["wp"] = wp
    res = run_bass_kernel_spmd(ncb, in_maps, list(range(NCORES)),
                               trace=trace, tmpdir=tmpdirs[0])
    out = np.zeros((BT, C), dtype=np.float32)
    for i in range(NCORES):
        out += res.results[i]["partial"]
    out += b_proj[None, :]
    return out.reshape(B, T, C), res


def kernel(x, w_qkv, w_proj, b_proj):
    out, _ = run(x, w_qkv, w_proj, b_proj)
    return out
